# revision 7
# baseline (speedup 1.0000x reference)
"""ActorGCN (GCNConv + BatchNorm + Linear + softmax) on 8 TRN2 NeuronCores.

Strategy
--------
The reference aggregates messages at HID=500. Since GCN aggregation commutes
with the linear transform, we aggregate at F_IN=20 instead (25x less traffic):
    agg_feat = D^-1/2 (A+I) D^-1/2 X          [N, 20]
BatchNorm batch statistics over the 500 hidden columns reduce exactly to the
20-dim first moment m and 20x20 second moment C of agg_feat:
    mean = m/N @ W,  var = W^T (C/N) W - mean0^2   (gcn_b cancels exactly)
and BN+Linear fold into a single [21, 2] matrix applied per node.

Sharding: nodes (and their in-edges) are partitioned by target across the 8
cores; each core's targets are split into 3 "thirds" so the per-third distinct
source set fits int16 dma_gather indices (compact halo tables). The only
irregular operation — gathering Y[src] per edge — runs via dma_gather
(SWDGE, 4 queues, 2048 idx/instruction, 256 B rows) over a JDS (jagged
diagonal) slot layout; per-pass accumulation is plain affine DVE adds.
BatchNorm statistics are combined with a single small AllReduce.

The Bass graph is built per-input (slot layout constants are baked in), which
is fine since kernel() compiles and runs in one call.
"""
import numpy as np

import concourse.bass as bass
import concourse.tile as tile
from concourse import bacc, mybir
from concourse.bass_utils import run_bass_kernel_spmd
from concourse.masks import make_identity

N = 50000
F = 20
HID = 500
OUT = 2
P = 128
N_CORES = 8
PER_CORE = N // N_CORES
N_THIRDS = 3
THIRD_SIZES = (2084, 2084, 2082)
ACOLS = 17                  # ceil(max third size / 128)
EW = 64                     # gather table row width in f32 (256 B)
CHUNK = 16                  # gather columns per dma_gather (16*128 = 2048 idx)
NQ = 4                      # SWDGE queues
BN_EPS = 1e-5
F32 = mybir.dt.float32
I16 = mybir.dt.int16

AF = mybir.ActivationFunctionType
OP = mybir.AluOpType


def _third_base(c, t3):
    return c * PER_CORE + sum(THIRD_SIZES[:t3])


def _preprocess(node_feature, edge_index):
    """Host-side index preprocessing: sharding, JDS layout, compact tables."""
    x = np.ascontiguousarray(np.asarray(node_feature, np.float32))
    ei = np.asarray(edge_index)
    row, col = ei[0].astype(np.int64), ei[1].astype(np.int64)
    deg = (np.bincount(col, minlength=N) + 1).astype(np.float32)

    order = np.argsort(col, kind="stable")
    row_s = row[order]
    starts = np.searchsorted(col[order], np.arange(N + 1))

    # per (core, third): ranked jagged lists
    thirds = [[None] * N_THIRDS for _ in range(N_CORES)]
    for c in range(N_CORES):
        for t3 in range(N_THIRDS):
            base = _third_base(c, t3)
            sz = THIRD_SIZES[t3]
            tgts = np.arange(base, base + sz)
            cnt = (starts[base + 1:base + sz + 1] - starts[base:base + sz])
            L = cnt + 1                      # + self loop
            Lmax = int(L.max())
            LL = np.full((sz, Lmax), -1, np.int64)
            mask = np.arange(Lmax)[None, :] < cnt[:, None]
            LL[mask] = row_s[starts[base]:starts[base + sz]]
            LL[np.arange(sz), cnt] = tgts    # self loop last
            key = L.astype(np.int64).copy()
            if c == 0 and t3 == 0:
                key[0] = 1 << 40             # node 0 -> rank 0
            rank_order = np.argsort(-key, kind="stable")
            thirds[c][t3] = dict(tgts=tgts, L=L, LL=LL, rank_order=rank_order)

    # uniform pass widths across cores (per third)
    ws_all, segs_all, C3 = [], [], []
    for t3 in range(N_THIRDS):
        Lmax_g = 0
        for c in range(N_CORES):
            th = thirds[c][t3]
            Le = th["L"][th["rank_order"]].copy()
            if c == 0 and t3 == 0:
                Le = np.maximum(Le, 1)       # node0 real length is enough here
            Lmax_g = max(Lmax_g, int(Le.max()))
        ws = []
        for d in range(Lmax_g):
            m = 0
            for c in range(N_CORES):
                th = thirds[c][t3]
                Le = th["L"][th["rank_order"]].astype(np.int64).copy()
                if c == 0 and t3 == 0:
                    Le[0] = 1 << 40          # node0 occupies every pass
                m = max(m, int((Le > d).sum()))
            ws.append((m + P - 1) // P)
        C = sum(ws)
        Cpad = ((C + CHUNK - 1) // CHUNK) * CHUNK
        ws_all.append(ws)
        C3.append(Cpad)
        # chunk -> list of (lo, hi, acc_lo): chunk-local col range -> acc cols
        cbs = np.cumsum([0] + ws)
        segs = []
        for c0 in range(0, Cpad, CHUNK):
            sl = []
            for d, w in enumerate(ws):
                lo = max(cbs[d], c0)
                hi = min(cbs[d] + w, c0 + CHUNK)
                if lo < hi:
                    sl.append((lo - c0, hi - c0, lo - cbs[d]))
            segs.append(sl)
        segs_all.append(segs)
    CT = sum(C3)

    # per-core slot arrays + compact tables
    n_tab = 0
    slot_src = {}
    for c in range(N_CORES):
        for t3 in range(N_THIRDS):
            th = thirds[c][t3]
            ws = ws_all[t3]
            sz = len(th["tgts"])
            LLr = th["LL"][th["rank_order"]]          # [sz, Lmax]
            Lr = th["L"][th["rank_order"]].astype(np.int64).copy()
            if c == 0 and t3 == 0:
                Lr0 = Lr.copy()                        # real lengths
            src = np.full((P, C3[t3]), -1, np.int64)
            cb = 0
            for d, w in enumerate(ws):
                if d < LLr.shape[1]:
                    rr = np.arange(w * P)
                    pp, cc = rr % P, rr // P
                    valid = (rr < sz) & (d < Lr[np.minimum(rr, sz - 1)])
                    rv = rr[valid]
                    if rv.size:
                        src[pp[valid], cb + cc[valid]] = LLr[rv, d]
                cb += w
            uniq = np.unique(src[src >= 0])
            assert len(uniq) + 1 <= 32767, f"table overflow {len(uniq)}"
            n_tab = max(n_tab, len(uniq) + 1)
            slot_src[(c, t3)] = (src, uniq)

    in_maps = []
    meta = dict(ws=ws_all, segs=segs_all, C3=C3, CT=CT, n_tab=n_tab,
                node_of_rank=[], deg=deg)
    for c in range(N_CORES):
        m = {}
        idx_cols = []
        deg_cols = []
        for t3 in range(N_THIRDS):
            src, uniq = slot_src[(c, t3)]
            lut = np.full(N, n_tab - 1, np.int64)
            lut[uniq] = np.arange(len(uniq))
            idx = np.where(src >= 0, lut[np.maximum(src, 0)], n_tab - 1)
            dslot = np.where(src >= 0, deg[np.maximum(src, 0)], 1.0)
            tab = np.zeros((n_tab, EW), np.float32)
            tab[: len(uniq), :F] = x[uniq]
            m[f"tab{t3}"] = tab
            idx_cols.append(idx.astype(np.int16))
            deg_cols.append(dslot.astype(np.float32))
        idx_full = np.concatenate(idx_cols, axis=1)     # [128, CT]
        # wrapped-16 + replicated layout per 2048-idx block
        blocks = []
        for b in range(CT // CHUNK):
            flat = idx_full[:, b * CHUNK:(b + 1) * CHUNK]
            # flat idx j = c*128 + p  ->  value flat[p, c]
            fj = flat.T.reshape(-1)                     # j order
            wrapped = fj.reshape(-1, 16).T              # [16, 128]
            blocks.append(np.tile(wrapped, (8, 1)))     # [128, 128]
        m["idx"] = np.concatenate(blocks, axis=1).astype(np.int16)
        m["degs"] = np.concatenate(deg_cols, axis=1)
        dn = np.ones((P, N_THIRDS * ACOLS), np.float32)
        nor = []
        for t3 in range(N_THIRDS):
            th = thirds[c][t3]
            sz = len(th["tgts"])
            r = np.arange(sz)
            dn[r % P, t3 * ACOLS + r // P] = deg[th["tgts"][th["rank_order"]]]
            nor.append(th["rank_order"])
        m["degn"] = dn
        meta["node_of_rank"].append(nor)
        in_maps.append(m)
    return in_maps, meta


def _build(meta, weights):
    ws_all, segs_all, C3, CT, n_tab = (
        meta["ws"], meta["segs"], meta["C3"], meta["CT"], meta["n_tab"])
    IDXW = CT * 8
    nc = bacc.Bacc("TRN2", target_bir_lowering=False, debug=False,
                   num_devices=N_CORES, num_swdge_queues=NQ)
    tabs = [nc.dram_tensor(f"tab{t}", [n_tab, EW], F32, kind="ExternalInput")
            for t in range(N_THIRDS)]
    idx_in = nc.dram_tensor("idx", [P, IDXW], I16, kind="ExternalInput")
    degs_in = nc.dram_tensor("degs", [P, CT], F32, kind="ExternalInput")
    degn_in = nc.dram_tensor("degn", [P, N_THIRDS * ACOLS], F32,
                             kind="ExternalInput")
    gw_in = nc.dram_tensor("gw", [F, HID], F32, kind="ExternalInput")
    gamma_in = nc.dram_tensor("gamma", [1, HID], F32, kind="ExternalInput")
    beta_in = nc.dram_tensor("beta", [1, HID], F32, kind="ExternalInput")
    lw_in = nc.dram_tensor("lw", [125, 8], F32, kind="ExternalInput")
    lb_in = nc.dram_tensor("lb", [1, OUT], F32, kind="ExternalInput")
    probs_out = nc.dram_tensor("probs", [P, N_THIRDS * ACOLS * OUT], F32,
                               kind="ExternalOutput")
    rsu_out = nc.dram_tensor("rsu", [1, HID], F32, kind="ExternalOutput")

    with tile.TileContext(nc) as tc:
        with (
            tc.tile_pool(name="const", bufs=1) as const,
            tc.tile_pool(name="gath", bufs=16) as gath,
            tc.tile_pool(name="work", bufs=4) as work,
            tc.tile_pool(name="accp", bufs=1) as accp,
            tc.tile_pool(name="ps", bufs=1, space="PSUM") as ps,
            tc.tile_pool(name="ps2", bufs=2, space="PSUM") as ps2,
            tc.tile_pool(name="dram", bufs=1, space="DRAM") as dram,
        ):
            # idx tile loaded per third so the first gathers start early
            idx_t = const.tile([P, IDXW], I16)
            degs_t = const.tile([P, CT], F32)
            ib = 0
            for t3 in range(N_THIRDS):
                w8 = C3[t3] * 8
                nc.sync.dma_start(idx_t[:, ib * 8:ib * 8 + w8],
                                  idx_in[:, ib * 8:ib * 8 + w8])
                nc.sync.dma_start(degs_t[:, ib:ib + C3[t3]],
                                  degs_in[:, ib:ib + C3[t3]])
                ib += C3[t3]
            degn_t = const.tile([P, N_THIRDS * ACOLS], F32)
            nc.sync.dma_start(degn_t[:], degn_in[:])

            # dinv of slot sources and of target nodes
            dinv_s = const.tile([P, CT], F32)
            nc.scalar.activation(dinv_s[:], degs_t[:], AF.Sqrt)
            nc.vector.reciprocal(dinv_s[:], dinv_s[:])
            dinv_n = const.tile([P, N_THIRDS * ACOLS], F32)
            nc.scalar.activation(dinv_n[:], degn_t[:], AF.Sqrt)
            nc.vector.reciprocal(dinv_n[:], dinv_n[:])

            acc3 = []
            for t3 in range(N_THIRDS):
                a = accp.tile([P, ACOLS * (F + 1)], F32, tag=f"acc{t3}")
                nc.vector.memset(a[:], 0.0)
                av = a[:].rearrange("p (c k) -> p c k", k=F + 1)
                nc.vector.memset(av[:, :, F:F + 1], 1.0)
                acc3.append(a)
            ident = const.tile([P, P], F32)
            make_identity(nc, ident[:])

            # ---- gather + scale + JDS accumulate; per-third stats on PE ----
            cp = ps.tile([F, F + 1], F32, tag="psC")
            aT = []            # transposed acc columns for the logits matmuls
            kmm = 0
            nmm = N_THIRDS * ACOLS
            qrr = 0
            gcol = 0
            for t3 in range(N_THIRDS):
                av = acc3[t3][:].rearrange("p (c k) -> p c k", k=F + 1)
                for ch in range(C3[t3] // CHUNK):
                    g = gath.tile([P, CHUNK * EW], F32, tag="g")
                    gv = g[:].rearrange("p (c e) -> p c e", c=CHUNK)
                    blk = (gcol // CHUNK) + ch
                    nc.gpsimd.dma_gather(
                        out_ap=gv,
                        in_ap=tabs[t3][:],
                        idxs_ap=idx_t[:, blk * 128:(blk + 1) * 128],
                        num_idxs=CHUNK * P,
                        num_idxs_reg=CHUNK * P,
                        elem_size=EW,
                        single_packet=False,
                        queue_num=qrr % NQ,
                    )
                    qrr += 1
                    sc = work.tile([P, CHUNK * F], F32, tag="sc")
                    scv = sc[:].rearrange("p (c f) -> p c f", c=CHUNK)
                    c0 = ch * CHUNK
                    dv = dinv_s[:, gcol + c0:gcol + c0 + CHUNK].to_broadcast(
                        [P, CHUNK, F])
                    nc.vector.tensor_tensor(
                        out=scv, in0=gv[:, :, 0:F], in1=dv, op=OP.mult)
                    for (lo, hi, alo) in segs_all[t3][ch]:
                        w = hi - lo
                        nc.vector.tensor_tensor(
                            out=av[:, alo:alo + w, 0:F],
                            in0=av[:, alo:alo + w, 0:F],
                            in1=scv[:, lo:hi, :],
                            op=OP.add)
                gcol += C3[t3]
                # this third's acc is final: scale by target dinv, then
                # stats matmuls + per-column transposes overlap later gathers
                dn = dinv_n[:, t3 * ACOLS:(t3 + 1) * ACOLS].to_broadcast(
                    [P, ACOLS, F])
                nc.vector.tensor_tensor(out=av[:, :, 0:F], in0=av[:, :, 0:F],
                                        in1=dn, op=OP.mult)
                for j in range(ACOLS):
                    nc.tensor.matmul(out=cp[:], lhsT=av[:, j, 0:F],
                                     rhs=av[:, j, 0:F + 1],
                                     start=(kmm == 0), stop=(kmm == nmm - 1))
                    kmm += 1
                    tps = ps2.tile([F + 1, P], F32, tag="psT")
                    nc.tensor.transpose(out=tps[:], in_=av[:, j, 0:F + 1],
                                        identity=ident[:])
                    ts = const.tile([F + 1, P], F32, tag=f"aT{t3}_{j}")
                    nc.vector.tensor_copy(out=ts[:], in_=tps[:])
                    aT.append(ts)
            cs = const.tile([F, F + 1], F32)
            nc.vector.tensor_copy(out=cs[:], in_=cp[:])

            # weights needed only for the epilogue
            gw_t = const.tile([F, HID], F32)
            nc.sync.dma_start(gw_t[:], gw_in[:])
            gamma_t = const.tile([1, HID], F32)
            nc.sync.dma_start(gamma_t[:], gamma_in[:])
            beta_t = const.tile([1, HID], F32)
            nc.sync.dma_start(beta_t[:], beta_in[:])
            lw_t = const.tile([125, 8], F32)
            nc.sync.dma_start(lw_t[:], lw_in[:])
            lb_t = const.tile([1, OUT], F32)
            nc.sync.dma_start(lb_t[:], lb_in[:])

            # ---- AllReduce of C' ----
            bin_ = dram.tile([F, F + 1], F32)
            bout = dram.tile([F, F + 1], F32, addr_space="Shared")
            nc.gpsimd.dma_start(bin_[:], cs[:])
            nc.gpsimd.collective_compute(
                "AllReduce", OP.add,
                replica_groups=[list(range(N_CORES))],
                ins=[bin_[:].opt()], outs=[bout[:].opt()])
            cg = const.tile([F, F + 1], F32)
            nc.sync.dma_start(cg[:], bout[:])

            # ---- fold BN into per-node [21, 2] matrix ----
            cn = const.tile([F, F + 1], F32)
            nc.vector.tensor_scalar_mul(cn[:], cg[:], 1.0 / N)
            gps = ps.tile([F, HID], F32, tag="psA")
            nc.tensor.matmul(out=gps[:], lhsT=cn[:, 0:F], rhs=gw_t[:],
                             start=True, stop=True)
            wg = const.tile([F, HID], F32)
            nc.vector.tensor_tensor(out=wg[:], in0=gw_t[:], in1=gps[:],
                                    op=OP.mult)
            m0ps = ps.tile([1, HID], F32, tag="psB")
            nc.tensor.matmul(out=m0ps[:], lhsT=cn[:, F:F + 1], rhs=gw_t[:],
                             start=True, stop=True)
            m0 = const.tile([1, HID], F32)
            nc.vector.tensor_copy(out=m0[:], in_=m0ps[:])
            ones20 = const.tile([F, 1], F32)
            nc.vector.memset(ones20[:], 1.0)
            e2ps = ps.tile([1, HID], F32, tag="psB")
            nc.tensor.matmul(out=e2ps[:], lhsT=ones20[:], rhs=wg[:],
                             start=True, stop=True)
            var = const.tile([1, HID], F32)
            nc.vector.tensor_tensor(out=var[:], in0=m0[:], in1=m0[:],
                                    op=OP.mult)
            nc.vector.tensor_tensor(out=var[:], in0=e2ps[:], in1=var[:],
                                    op=OP.subtract)
            nc.vector.tensor_scalar_add(var[:], var[:], BN_EPS)
            sd = const.tile([1, HID], F32)
            nc.scalar.activation(sd[:], var[:], AF.Sqrt)
            nc.vector.reciprocal(sd[:], sd[:])
            s_t = const.tile([1, HID], F32)
            nc.vector.tensor_tensor(out=s_t[:], in0=sd[:], in1=gamma_t[:],
                                    op=OP.mult)
            tvec = const.tile([1, HID], F32)
            nc.vector.tensor_tensor(out=tvec[:], in0=m0[:], in1=s_t[:],
                                    op=OP.mult)
            nc.vector.tensor_tensor(out=tvec[:], in0=beta_t[:], in1=tvec[:],
                                    op=OP.subtract)
            ones1x20 = const.tile([1, F], F32)
            nc.vector.memset(ones1x20[:], 1.0)
            sbps = ps.tile([F, HID], F32, tag="psA")
            nc.tensor.matmul(out=sbps[:], lhsT=ones1x20[:], rhs=s_t[:],
                             start=True, stop=True)
            ws_t = const.tile([F, HID], F32)
            nc.vector.tensor_tensor(out=ws_t[:], in0=gw_t[:], in1=sbps[:],
                                    op=OP.mult)
            ident = const.tile([P, P], F32)
            make_identity(nc, ident[:])
            wcbps = ps.tile([F + 1, OUT], F32, tag="psB3")
            for c4 in range(4):
                trp = ps2.tile([125, F], F32, tag="psT")
                nc.tensor.transpose(out=trp[:],
                                    in_=ws_t[:, c4 * 125:(c4 + 1) * 125],
                                    identity=ident[0:F, 0:F])
                trpt = ps2.tile([125, 1], F32, tag="psL")
                nc.tensor.transpose(out=trpt[:],
                                    in_=tvec[:, c4 * 125:(c4 + 1) * 125],
                                    identity=ident[0:1, 0:1])
                trs = work.tile([125, F + 1], F32, tag="trs")
                nc.vector.tensor_copy(out=trs[:, 0:F], in_=trp[:])
                nc.vector.tensor_copy(out=trs[:, F:F + 1], in_=trpt[:])
                nc.tensor.matmul(out=wcbps[:], lhsT=trs[:],
                                 rhs=lw_t[:, c4 * OUT:(c4 + 1) * OUT],
                                 start=(c4 == 0), stop=False)
            e20 = const.tile([1, F + 1], F32)
            nc.vector.memset(e20[:], 0.0)
            nc.vector.memset(e20[:, F:F + 1], 1.0)
            nc.tensor.matmul(out=wcbps[:], lhsT=e20[:], rhs=lb_t[:],
                             start=False, stop=True)
            wcb = const.tile([F + 1, OUT], F32)
            nc.vector.tensor_copy(out=wcb[:], in_=wcbps[:])

            # ---- logits on PE, relu, 2-class softmax ----
            probs_t = const.tile([P, N_THIRDS * ACOLS * OUT], F32)
            pv = probs_t[:].rearrange("p (t c o) -> p t c o", t=N_THIRDS,
                                      o=OUT)
            for t3 in range(N_THIRDS):
                for j in range(ACOLS):
                    lps = ps2.tile([P, OUT], F32, tag="psL")
                    nc.tensor.matmul(out=lps[:],
                                     lhsT=aT[t3 * ACOLS + j][:],
                                     rhs=wcb[:], start=True, stop=True)
                    nc.scalar.activation(pv[:, t3, j, :], lps[:], AF.Relu)
            for t3 in range(N_THIRDS):
                d = work.tile([P, ACOLS], F32, tag="d")
                nc.vector.tensor_tensor(out=d[:], in0=pv[:, t3, :, 0],
                                        in1=pv[:, t3, :, 1], op=OP.subtract)
                nc.scalar.activation(pv[:, t3, :, 0], d[:], AF.Sigmoid)
                nc.scalar.activation(pv[:, t3, :, 1], d[:], AF.Sigmoid,
                                     scale=-1.0)
            nc.sync.dma_start(probs_out[:], probs_t[:])

            # ---- rsu embedding (meaningful on core 0 only) ----
            a0 = acc3[0][0:1, 0:F]                       # node0 at rank 0
            a0ps = ps.tile([F, 1], F32, tag="psB")
            nc.tensor.transpose(out=a0ps[:], in_=a0, identity=ident[0:1, 0:1])
            a0T = work.tile([F, 1], F32, tag="a0T")
            nc.vector.tensor_copy(out=a0T[:], in_=a0ps[:])
            rsups = ps.tile([1, HID], F32, tag="psA")
            nc.tensor.matmul(out=rsups[:], lhsT=a0T[:], rhs=ws_t[:],
                             start=True, stop=True)
            rsu_t = const.tile([1, HID], F32)
            nc.vector.tensor_tensor(out=rsu_t[:], in0=rsups[:], in1=tvec[:],
                                    op=OP.add)
            nc.sync.dma_start(rsu_out[:], rsu_t[:])
    nc.finalize()
    return nc


def _run(inputs, trace=False):
    node_feature = np.asarray(inputs["node_feature"], np.float32)
    edge_index = np.asarray(inputs["edge_index"])
    gcn_W = np.asarray(inputs["gcn_W"], np.float32)
    bn_gamma = np.asarray(inputs["bn_gamma"], np.float32)
    bn_beta = np.asarray(inputs["bn_beta"], np.float32)
    lin_W = np.asarray(inputs["lin_W"], np.float32)
    lin_b = np.asarray(inputs["lin_b"], np.float32)

    in_maps, meta = _preprocess(node_feature, edge_index)
    lw4 = np.ascontiguousarray(
        lin_W.reshape(4, 125, OUT).transpose(1, 0, 2).reshape(125, 4 * OUT))
    for m in in_maps:
        m["gw"] = np.ascontiguousarray(gcn_W)
        m["gamma"] = np.ascontiguousarray(bn_gamma[None, :])
        m["beta"] = np.ascontiguousarray(bn_beta[None, :])
        m["lw"] = lw4
        m["lb"] = np.ascontiguousarray(lin_b[None, :])

    nc = _build(meta, None)
    res = run_bass_kernel_spmd(nc, in_maps, core_ids=list(range(N_CORES)),
                               trace=trace)

    prob_full = np.zeros((N, OUT), np.float32)
    for c in range(N_CORES):
        probs = res.results[c]["probs"].reshape(P, N_THIRDS, ACOLS, OUT)
        for t3 in range(N_THIRDS):
            base = _third_base(c, t3)
            sz = THIRD_SIZES[t3]
            nor = meta["node_of_rank"][c][t3]
            r = np.arange(sz)
            prob_full[base + nor] = probs[r % P, t3, r // P]
    rsu = res.results[0]["rsu"].reshape(1, HID)
    return (prob_full, rsu), res


def kernel(**inputs):
    (prob, rsu), _ = _run(inputs, trace=False)
    return prob, rsu


# revision 8
# speedup vs baseline: 1.0779x; 1.0779x over previous
"""ActorGCN (GCNConv + BatchNorm + Linear + softmax) on 8 TRN2 NeuronCores.

Strategy
--------
The reference aggregates messages at HID=500. Since GCN aggregation commutes
with the linear transform, we aggregate at F_IN=20 instead (25x less traffic):
    agg_feat = D^-1/2 (A+I) D^-1/2 X          [N, 20]
BatchNorm batch statistics over the 500 hidden columns reduce exactly to the
20-dim first moment m and 20x20 second moment C of agg_feat:
    mean = m/N @ W,  var = W^T (C/N) W - mean0^2   (gcn_b cancels exactly)
and BN+Linear fold into a single [21, 2] matrix applied per node.

Sharding: nodes (and their in-edges) are partitioned by target across the 8
cores; each core's targets are split into 3 "thirds" so the per-third distinct
source set fits int16 dma_gather indices (compact halo tables). The only
irregular operation — gathering Y[src] per edge — runs via dma_gather
(SWDGE, 4 queues, 2048 idx/instruction, 256 B rows) over a JDS (jagged
diagonal) slot layout; per-pass accumulation is plain affine DVE adds.
BatchNorm statistics are combined with a single small AllReduce.

The Bass graph is built per-input (slot layout constants are baked in), which
is fine since kernel() compiles and runs in one call.
"""
import numpy as np

import concourse.bass as bass
import concourse.tile as tile
from concourse import bacc, mybir
from concourse.bass_utils import run_bass_kernel_spmd
from concourse.masks import make_identity

N = 50000
F = 20
HID = 500
OUT = 2
P = 128
N_CORES = 8
PER_CORE = N // N_CORES
N_THIRDS = 3
THIRD_SIZES = (2084, 2084, 2082)
ACOLS = 17                  # ceil(max third size / 128)
EW = 64                     # gather table row width in f32 (256 B)
CHUNK = 16                  # gather columns per dma_gather (16*128 = 2048 idx)
NQ = 4                      # SWDGE queues
BN_EPS = 1e-5
F32 = mybir.dt.float32
I16 = mybir.dt.int16

AF = mybir.ActivationFunctionType
OP = mybir.AluOpType


def _third_base(c, t3):
    return c * PER_CORE + sum(THIRD_SIZES[:t3])


def _preprocess(node_feature, edge_index):
    """Host-side index preprocessing: sharding, JDS layout, compact tables."""
    x = np.ascontiguousarray(np.asarray(node_feature, np.float32))
    ei = np.asarray(edge_index)
    row, col = ei[0].astype(np.int64), ei[1].astype(np.int64)
    deg = (np.bincount(col, minlength=N) + 1).astype(np.float32)

    order = np.argsort(col, kind="stable")
    row_s = row[order]
    starts = np.searchsorted(col[order], np.arange(N + 1))

    # per (core, third): ranked jagged lists
    thirds = [[None] * N_THIRDS for _ in range(N_CORES)]
    for c in range(N_CORES):
        for t3 in range(N_THIRDS):
            base = _third_base(c, t3)
            sz = THIRD_SIZES[t3]
            tgts = np.arange(base, base + sz)
            cnt = (starts[base + 1:base + sz + 1] - starts[base:base + sz])
            L = cnt + 1                      # + self loop
            Lmax = int(L.max())
            LL = np.full((sz, Lmax), -1, np.int64)
            mask = np.arange(Lmax)[None, :] < cnt[:, None]
            LL[mask] = row_s[starts[base]:starts[base + sz]]
            LL[np.arange(sz), cnt] = tgts    # self loop last
            key = L.astype(np.int64).copy()
            if c == 0 and t3 == 0:
                key[0] = 1 << 40             # node 0 -> rank 0
            rank_order = np.argsort(-key, kind="stable")
            thirds[c][t3] = dict(tgts=tgts, L=L, LL=LL, rank_order=rank_order)

    # uniform pass widths across cores (per third)
    ws_all, segs_all, C3 = [], [], []
    for t3 in range(N_THIRDS):
        Lmax_g = 0
        for c in range(N_CORES):
            th = thirds[c][t3]
            Le = th["L"][th["rank_order"]].copy()
            if c == 0 and t3 == 0:
                Le = np.maximum(Le, 1)       # node0 real length is enough here
            Lmax_g = max(Lmax_g, int(Le.max()))
        ws = []
        for d in range(Lmax_g):
            m = 0
            for c in range(N_CORES):
                th = thirds[c][t3]
                Le = th["L"][th["rank_order"]].astype(np.int64).copy()
                if c == 0 and t3 == 0:
                    Le[0] = 1 << 40          # node0 occupies every pass
                m = max(m, int((Le > d).sum()))
            ws.append((m + P - 1) // P)
        C = sum(ws)
        Cpad = ((C + CHUNK - 1) // CHUNK) * CHUNK
        ws_all.append(ws)
        C3.append(Cpad)
        # chunk -> list of (lo, hi, acc_lo): chunk-local col range -> acc cols
        cbs = np.cumsum([0] + ws)
        segs = []
        for c0 in range(0, Cpad, CHUNK):
            sl = []
            for d, w in enumerate(ws):
                lo = max(cbs[d], c0)
                hi = min(cbs[d] + w, c0 + CHUNK)
                if lo < hi:
                    sl.append((lo - c0, hi - c0, lo - cbs[d]))
            segs.append(sl)
        segs_all.append(segs)
    CT = sum(C3)

    # per-core slot arrays + compact tables
    n_tab = 0
    slot_src = {}
    for c in range(N_CORES):
        for t3 in range(N_THIRDS):
            th = thirds[c][t3]
            ws = ws_all[t3]
            sz = len(th["tgts"])
            LLr = th["LL"][th["rank_order"]]          # [sz, Lmax]
            Lr = th["L"][th["rank_order"]].astype(np.int64).copy()
            if c == 0 and t3 == 0:
                Lr0 = Lr.copy()                        # real lengths
            src = np.full((P, C3[t3]), -1, np.int64)
            cb = 0
            for d, w in enumerate(ws):
                if d < LLr.shape[1]:
                    rr = np.arange(w * P)
                    pp, cc = rr % P, rr // P
                    valid = (rr < sz) & (d < Lr[np.minimum(rr, sz - 1)])
                    rv = rr[valid]
                    if rv.size:
                        src[pp[valid], cb + cc[valid]] = LLr[rv, d]
                cb += w
            uniq = np.unique(src[src >= 0])
            assert len(uniq) + 1 <= 32767, f"table overflow {len(uniq)}"
            n_tab = max(n_tab, len(uniq) + 1)
            slot_src[(c, t3)] = (src, uniq)

    in_maps = []
    meta = dict(ws=ws_all, segs=segs_all, C3=C3, CT=CT, n_tab=n_tab,
                node_of_rank=[], deg=deg)
    for c in range(N_CORES):
        m = {}
        idx_cols = []
        deg_cols = []
        for t3 in range(N_THIRDS):
            src, uniq = slot_src[(c, t3)]
            lut = np.full(N, n_tab - 1, np.int64)
            lut[uniq] = np.arange(len(uniq))
            idx = np.where(src >= 0, lut[np.maximum(src, 0)], n_tab - 1)
            dslot = np.where(src >= 0, deg[np.maximum(src, 0)], 1.0)
            tab = np.zeros((n_tab, EW), np.float32)
            tab[: len(uniq), :F] = x[uniq]
            m[f"tab{t3}"] = tab
            idx_cols.append(idx.astype(np.int16))
            deg_cols.append(dslot.astype(np.float32))
        idx_full = np.concatenate(idx_cols, axis=1)     # [128, CT]
        # wrapped-16 + replicated layout per 2048-idx block
        blocks = []
        for b in range(CT // CHUNK):
            flat = idx_full[:, b * CHUNK:(b + 1) * CHUNK]
            # flat idx j = c*128 + p  ->  value flat[p, c]
            fj = flat.T.reshape(-1)                     # j order
            wrapped = fj.reshape(-1, 16).T              # [16, 128]
            blocks.append(np.tile(wrapped, (8, 1)))     # [128, 128]
        m["idx"] = np.concatenate(blocks, axis=1).astype(np.int16)
        m["degs"] = np.concatenate(deg_cols, axis=1)
        dn = np.ones((P, N_THIRDS * ACOLS), np.float32)
        nor = []
        for t3 in range(N_THIRDS):
            th = thirds[c][t3]
            sz = len(th["tgts"])
            r = np.arange(sz)
            dn[r % P, t3 * ACOLS + r // P] = deg[th["tgts"][th["rank_order"]]]
            nor.append(th["rank_order"])
        m["degn"] = dn
        meta["node_of_rank"].append(nor)
        in_maps.append(m)
    return in_maps, meta


def _build(meta, weights):
    ws_all, segs_all, C3, CT, n_tab = (
        meta["ws"], meta["segs"], meta["C3"], meta["CT"], meta["n_tab"])
    IDXW = CT * 8
    nc = bacc.Bacc("TRN2", target_bir_lowering=False, debug=False,
                   num_devices=N_CORES, num_swdge_queues=NQ)
    tabs = [nc.dram_tensor(f"tab{t}", [n_tab, EW], F32, kind="ExternalInput")
            for t in range(N_THIRDS)]
    idx_in = nc.dram_tensor("idx", [P, IDXW], I16, kind="ExternalInput")
    degs_in = nc.dram_tensor("degs", [P, CT], F32, kind="ExternalInput")
    degn_in = nc.dram_tensor("degn", [P, N_THIRDS * ACOLS], F32,
                             kind="ExternalInput")
    gw_in = nc.dram_tensor("gw", [F, HID], F32, kind="ExternalInput")
    gamma_in = nc.dram_tensor("gamma", [1, HID], F32, kind="ExternalInput")
    beta_in = nc.dram_tensor("beta", [1, HID], F32, kind="ExternalInput")
    lw_in = nc.dram_tensor("lw", [125, 8], F32, kind="ExternalInput")
    lb_in = nc.dram_tensor("lb", [1, OUT], F32, kind="ExternalInput")
    probs_out = nc.dram_tensor("probs", [P, N_THIRDS * ACOLS * OUT], F32,
                               kind="ExternalOutput")
    rsu_out = nc.dram_tensor("rsu", [1, HID], F32, kind="ExternalOutput")

    with tile.TileContext(nc) as tc:
        with (
            tc.tile_pool(name="const", bufs=1) as const,
            tc.tile_pool(name="gath", bufs=16) as gath,
            tc.tile_pool(name="work", bufs=4) as work,
            tc.tile_pool(name="accp", bufs=1) as accp,
            tc.tile_pool(name="ps", bufs=1, space="PSUM") as ps,
            tc.tile_pool(name="ps2", bufs=2, space="PSUM") as ps2,
            tc.tile_pool(name="dram", bufs=1, space="DRAM") as dram,
        ):
            # warmup: tiny gather to pull the SWDGE gather ucode into IRAM
            wu_idx = const.tile([P, 8], I16)
            nc.vector.memset(wu_idx[:], 0)
            wu_out = const.tile([P, EW], F32)
            nc.gpsimd.dma_gather(
                out_ap=wu_out[:].rearrange("p (c e) -> p c e", c=1),
                in_ap=tabs[0][:], idxs_ap=wu_idx[:], num_idxs=P,
                num_idxs_reg=P, elem_size=EW, single_packet=False,
                queue_num=0)
            # idx tile loaded per third so the first gathers start early
            idx_t = const.tile([P, IDXW], I16)
            degs_t = const.tile([P, CT], F32)
            ib = 0
            for t3 in range(N_THIRDS):
                w8 = C3[t3] * 8
                nc.sync.dma_start(idx_t[:, ib * 8:ib * 8 + w8],
                                  idx_in[:, ib * 8:ib * 8 + w8])
                nc.sync.dma_start(degs_t[:, ib:ib + C3[t3]],
                                  degs_in[:, ib:ib + C3[t3]])
                ib += C3[t3]
            degn_t = const.tile([P, N_THIRDS * ACOLS], F32)
            nc.sync.dma_start(degn_t[:], degn_in[:])

            # dinv of slot sources and of target nodes
            dinv_s = const.tile([P, CT], F32)
            nc.scalar.activation(dinv_s[:], degs_t[:], AF.Sqrt)
            nc.vector.reciprocal(dinv_s[:], dinv_s[:])
            dinv_n = const.tile([P, N_THIRDS * ACOLS], F32)
            nc.scalar.activation(dinv_n[:], degn_t[:], AF.Sqrt)
            nc.vector.reciprocal(dinv_n[:], dinv_n[:])

            acc3 = []
            for t3 in range(N_THIRDS):
                a = accp.tile([P, ACOLS * (F + 1)], F32, tag=f"acc{t3}")
                nc.vector.memset(a[:], 0.0)
                av = a[:].rearrange("p (c k) -> p c k", k=F + 1)
                nc.vector.memset(av[:, :, F:F + 1], 1.0)
                acc3.append(a)
            ident = const.tile([P, P], F32)
            make_identity(nc, ident[:])

            # ---- gather + scale + JDS accumulate; per-third stats on PE ----
            aT = []            # transposed acc columns for the logits matmuls
            cs3 = []
            qrr = 0
            gcol = 0
            for t3 in range(N_THIRDS):
                av = acc3[t3][:].rearrange("p (c k) -> p c k", k=F + 1)
                for ch in range(C3[t3] // CHUNK):
                    g = gath.tile([P, CHUNK * EW], F32, tag="g")
                    gv = g[:].rearrange("p (c e) -> p c e", c=CHUNK)
                    blk = (gcol // CHUNK) + ch
                    nc.gpsimd.dma_gather(
                        out_ap=gv,
                        in_ap=tabs[t3][:],
                        idxs_ap=idx_t[:, blk * 128:(blk + 1) * 128],
                        num_idxs=CHUNK * P,
                        num_idxs_reg=CHUNK * P,
                        elem_size=EW,
                        single_packet=False,
                        queue_num=qrr % NQ,
                    )
                    qrr += 1
                    sc = work.tile([P, CHUNK * F], F32, tag="sc")
                    scv = sc[:].rearrange("p (c f) -> p c f", c=CHUNK)
                    c0 = ch * CHUNK
                    dv = dinv_s[:, gcol + c0:gcol + c0 + CHUNK].to_broadcast(
                        [P, CHUNK, F])
                    nc.vector.tensor_tensor(
                        out=scv, in0=gv[:, :, 0:F], in1=dv, op=OP.mult)
                    for (lo, hi, alo) in segs_all[t3][ch]:
                        w = hi - lo
                        nc.vector.tensor_tensor(
                            out=av[:, alo:alo + w, 0:F],
                            in0=av[:, alo:alo + w, 0:F],
                            in1=scv[:, lo:hi, :],
                            op=OP.add)
                gcol += C3[t3]
                # this third's acc is final: scale by target dinv, then
                # stats matmuls + per-column transposes overlap later gathers
                dn = dinv_n[:, t3 * ACOLS:(t3 + 1) * ACOLS].to_broadcast(
                    [P, ACOLS, F])
                nc.vector.tensor_tensor(out=av[:, :, 0:F], in0=av[:, :, 0:F],
                                        in1=dn, op=OP.mult)
                cp = ps.tile([F, F + 1], F32, tag="psC")
                for j in range(ACOLS):
                    nc.tensor.matmul(out=cp[:], lhsT=av[:, j, 0:F],
                                     rhs=av[:, j, 0:F + 1],
                                     start=(j == 0), stop=(j == ACOLS - 1))
                    tps = ps2.tile([F + 1, P], F32, tag="psT")
                    nc.tensor.transpose(out=tps[:], in_=av[:, j, 0:F + 1],
                                        identity=ident[:])
                    ts = const.tile([F + 1, P], F32, tag=f"aT{t3}_{j}")
                    nc.vector.tensor_copy(out=ts[:], in_=tps[:])
                    aT.append(ts)
                cst = const.tile([F, F + 1], F32, tag=f"cs{t3}")
                nc.vector.tensor_copy(out=cst[:], in_=cp[:])
                cs3.append(cst)
            cs = const.tile([F, F + 1], F32)
            nc.vector.tensor_tensor(out=cs[:], in0=cs3[0][:], in1=cs3[1][:],
                                    op=OP.add)
            nc.vector.tensor_tensor(out=cs[:], in0=cs[:], in1=cs3[2][:],
                                    op=OP.add)

            # weights needed only for the epilogue
            gw_t = const.tile([F, HID], F32)
            nc.sync.dma_start(gw_t[:], gw_in[:])
            gamma_t = const.tile([1, HID], F32)
            nc.sync.dma_start(gamma_t[:], gamma_in[:])
            beta_t = const.tile([1, HID], F32)
            nc.sync.dma_start(beta_t[:], beta_in[:])
            lw_t = const.tile([125, 8], F32)
            nc.sync.dma_start(lw_t[:], lw_in[:])
            lb_t = const.tile([1, OUT], F32)
            nc.sync.dma_start(lb_t[:], lb_in[:])

            # ---- AllReduce of C' ----
            bin_ = dram.tile([F, F + 1], F32)
            bout = dram.tile([F, F + 1], F32, addr_space="Shared")
            nc.gpsimd.dma_start(bin_[:], cs[:])
            nc.gpsimd.collective_compute(
                "AllReduce", OP.add,
                replica_groups=[list(range(N_CORES))],
                ins=[bin_[:].opt()], outs=[bout[:].opt()])
            cg = const.tile([F, F + 1], F32)
            nc.sync.dma_start(cg[:], bout[:])

            # ---- fold BN into per-node [21, 2] matrix ----
            cn = const.tile([F, F + 1], F32)
            nc.vector.tensor_scalar_mul(cn[:], cg[:], 1.0 / N)
            gps = ps.tile([F, HID], F32, tag="psA")
            nc.tensor.matmul(out=gps[:], lhsT=cn[:, 0:F], rhs=gw_t[:],
                             start=True, stop=True)
            wg = const.tile([F, HID], F32)
            nc.vector.tensor_tensor(out=wg[:], in0=gw_t[:], in1=gps[:],
                                    op=OP.mult)
            m0ps = ps.tile([1, HID], F32, tag="psB")
            nc.tensor.matmul(out=m0ps[:], lhsT=cn[:, F:F + 1], rhs=gw_t[:],
                             start=True, stop=True)
            m0 = const.tile([1, HID], F32)
            nc.vector.tensor_copy(out=m0[:], in_=m0ps[:])
            ones20 = const.tile([F, 1], F32)
            nc.vector.memset(ones20[:], 1.0)
            e2ps = ps.tile([1, HID], F32, tag="psB")
            nc.tensor.matmul(out=e2ps[:], lhsT=ones20[:], rhs=wg[:],
                             start=True, stop=True)
            var = const.tile([1, HID], F32)
            nc.vector.tensor_tensor(out=var[:], in0=m0[:], in1=m0[:],
                                    op=OP.mult)
            nc.vector.tensor_tensor(out=var[:], in0=e2ps[:], in1=var[:],
                                    op=OP.subtract)
            nc.vector.tensor_scalar_add(var[:], var[:], BN_EPS)
            sd = const.tile([1, HID], F32)
            nc.scalar.activation(sd[:], var[:], AF.Sqrt)
            nc.vector.reciprocal(sd[:], sd[:])
            s_t = const.tile([1, HID], F32)
            nc.vector.tensor_tensor(out=s_t[:], in0=sd[:], in1=gamma_t[:],
                                    op=OP.mult)
            tvec = const.tile([1, HID], F32)
            nc.vector.tensor_tensor(out=tvec[:], in0=m0[:], in1=s_t[:],
                                    op=OP.mult)
            nc.vector.tensor_tensor(out=tvec[:], in0=beta_t[:], in1=tvec[:],
                                    op=OP.subtract)
            ones1x20 = const.tile([1, F], F32)
            nc.vector.memset(ones1x20[:], 1.0)
            sbps = ps.tile([F, HID], F32, tag="psA")
            nc.tensor.matmul(out=sbps[:], lhsT=ones1x20[:], rhs=s_t[:],
                             start=True, stop=True)
            ws_t = const.tile([F, HID], F32)
            nc.vector.tensor_tensor(out=ws_t[:], in0=gw_t[:], in1=sbps[:],
                                    op=OP.mult)
            ident = const.tile([P, P], F32)
            make_identity(nc, ident[:])
            wcbps = ps.tile([F + 1, OUT], F32, tag="psB3")
            for c4 in range(4):
                trp = ps2.tile([125, F], F32, tag="psT")
                nc.tensor.transpose(out=trp[:],
                                    in_=ws_t[:, c4 * 125:(c4 + 1) * 125],
                                    identity=ident[0:F, 0:F])
                trpt = ps2.tile([125, 1], F32, tag="psL")
                nc.tensor.transpose(out=trpt[:],
                                    in_=tvec[:, c4 * 125:(c4 + 1) * 125],
                                    identity=ident[0:1, 0:1])
                trs = work.tile([125, F + 1], F32, tag="trs")
                nc.vector.tensor_copy(out=trs[:, 0:F], in_=trp[:])
                nc.vector.tensor_copy(out=trs[:, F:F + 1], in_=trpt[:])
                nc.tensor.matmul(out=wcbps[:], lhsT=trs[:],
                                 rhs=lw_t[:, c4 * OUT:(c4 + 1) * OUT],
                                 start=(c4 == 0), stop=False)
            e20 = const.tile([1, F + 1], F32)
            nc.vector.memset(e20[:], 0.0)
            nc.vector.memset(e20[:, F:F + 1], 1.0)
            nc.tensor.matmul(out=wcbps[:], lhsT=e20[:], rhs=lb_t[:],
                             start=False, stop=True)
            wcb = const.tile([F + 1, OUT], F32)
            nc.vector.tensor_copy(out=wcb[:], in_=wcbps[:])

            # ---- logits on PE, relu, 2-class softmax ----
            probs_t = const.tile([P, N_THIRDS * ACOLS * OUT], F32)
            pv = probs_t[:].rearrange("p (t c o) -> p t c o", t=N_THIRDS,
                                      o=OUT)
            for t3 in range(N_THIRDS):
                lps = ps2.tile([P, ACOLS * OUT], F32, tag="psL")
                for j in range(ACOLS):
                    nc.tensor.matmul(out=lps[:, j * OUT:(j + 1) * OUT],
                                     lhsT=aT[t3 * ACOLS + j][:],
                                     rhs=wcb[:], start=True, stop=True)
                nc.scalar.activation(pv[:, t3, :, :].rearrange(
                    "p c o -> p (c o)"), lps[:], AF.Relu)
            for t3 in range(N_THIRDS):
                d = work.tile([P, ACOLS], F32, tag="d")
                nc.vector.tensor_tensor(out=d[:], in0=pv[:, t3, :, 0],
                                        in1=pv[:, t3, :, 1], op=OP.subtract)
                nc.scalar.activation(pv[:, t3, :, 0], d[:], AF.Sigmoid)
                nc.scalar.activation(pv[:, t3, :, 1], d[:], AF.Sigmoid,
                                     scale=-1.0)
            nc.sync.dma_start(probs_out[:], probs_t[:])

            # ---- rsu embedding (meaningful on core 0 only) ----
            a0 = acc3[0][0:1, 0:F]                       # node0 at rank 0
            a0ps = ps.tile([F, 1], F32, tag="psB")
            nc.tensor.transpose(out=a0ps[:], in_=a0, identity=ident[0:1, 0:1])
            a0T = work.tile([F, 1], F32, tag="a0T")
            nc.vector.tensor_copy(out=a0T[:], in_=a0ps[:])
            rsups = ps.tile([1, HID], F32, tag="psA")
            nc.tensor.matmul(out=rsups[:], lhsT=a0T[:], rhs=ws_t[:],
                             start=True, stop=True)
            rsu_t = const.tile([1, HID], F32)
            nc.vector.tensor_tensor(out=rsu_t[:], in0=rsups[:], in1=tvec[:],
                                    op=OP.add)
            nc.sync.dma_start(rsu_out[:], rsu_t[:])
    nc.finalize()
    return nc


def _run(inputs, trace=False):
    node_feature = np.asarray(inputs["node_feature"], np.float32)
    edge_index = np.asarray(inputs["edge_index"])
    gcn_W = np.asarray(inputs["gcn_W"], np.float32)
    bn_gamma = np.asarray(inputs["bn_gamma"], np.float32)
    bn_beta = np.asarray(inputs["bn_beta"], np.float32)
    lin_W = np.asarray(inputs["lin_W"], np.float32)
    lin_b = np.asarray(inputs["lin_b"], np.float32)

    in_maps, meta = _preprocess(node_feature, edge_index)
    lw4 = np.ascontiguousarray(
        lin_W.reshape(4, 125, OUT).transpose(1, 0, 2).reshape(125, 4 * OUT))
    for m in in_maps:
        m["gw"] = np.ascontiguousarray(gcn_W)
        m["gamma"] = np.ascontiguousarray(bn_gamma[None, :])
        m["beta"] = np.ascontiguousarray(bn_beta[None, :])
        m["lw"] = lw4
        m["lb"] = np.ascontiguousarray(lin_b[None, :])

    nc = _build(meta, None)
    res = run_bass_kernel_spmd(nc, in_maps, core_ids=list(range(N_CORES)),
                               trace=trace)

    prob_full = np.zeros((N, OUT), np.float32)
    for c in range(N_CORES):
        probs = res.results[c]["probs"].reshape(P, N_THIRDS, ACOLS, OUT)
        for t3 in range(N_THIRDS):
            base = _third_base(c, t3)
            sz = THIRD_SIZES[t3]
            nor = meta["node_of_rank"][c][t3]
            r = np.arange(sz)
            prob_full[base + nor] = probs[r % P, t3, r // P]
    rsu = res.results[0]["rsu"].reshape(1, HID)
    return (prob_full, rsu), res


def kernel(**inputs):
    (prob, rsu), _ = _run(inputs, trace=False)
    return prob, rsu


# revision 9
# speedup vs baseline: 1.2117x; 1.1241x over previous
"""ActorGCN (GCNConv + BatchNorm + Linear + softmax) on 8 TRN2 NeuronCores.

Strategy
--------
The reference aggregates messages at HID=500. Since GCN aggregation commutes
with the linear transform, we aggregate at F_IN=20 instead (25x less traffic):
    agg_feat = D^-1/2 (A+I) D^-1/2 X          [N, 20]
BatchNorm batch statistics over the 500 hidden columns reduce exactly to the
20-dim first moment m and 20x20 second moment C of agg_feat:
    mean = m/N @ W,  var = W^T (C/N) W - mean0^2   (gcn_b cancels exactly)
and BN+Linear fold into a single [21, 2] matrix applied per node.

Sharding: nodes (and their in-edges) are partitioned by target across the 8
cores; each core's targets are split into 3 "thirds" so the per-third distinct
source set fits int16 dma_gather indices (compact halo tables). The only
irregular operation — gathering Y[src] per edge — runs via dma_gather
(SWDGE, 4 queues, 2048 idx/instruction, 256 B rows) over a JDS (jagged
diagonal) slot layout; per-pass accumulation is plain affine DVE adds.
BatchNorm statistics are combined with a single small AllReduce.

The Bass graph is built per-input (slot layout constants are baked in), which
is fine since kernel() compiles and runs in one call.
"""
import numpy as np

import concourse.bass as bass
import concourse.tile as tile
from concourse import bacc, mybir
from concourse.bass_utils import run_bass_kernel_spmd
from concourse.masks import make_identity

N = 50000
F = 20
HID = 500
OUT = 2
P = 128
N_CORES = 8
PER_CORE = N // N_CORES
N_THIRDS = 3
THIRD_SIZES = (2084, 2084, 2082)
ACOLS = 17                  # ceil(max third size / 128)
EW = 64                     # gather table row width in f32 (256 B)
CHUNK = 16                  # gather columns per dma_gather (16*128 = 2048 idx)
NQ = 4                      # SWDGE queues
BN_EPS = 1e-5
F32 = mybir.dt.float32
I16 = mybir.dt.int16

AF = mybir.ActivationFunctionType
OP = mybir.AluOpType


def _third_base(c, t3):
    return c * PER_CORE + sum(THIRD_SIZES[:t3])


def _preprocess(node_feature, edge_index):
    """Host-side index preprocessing: sharding, pair-matching, JDS layout."""
    x = np.ascontiguousarray(np.asarray(node_feature, np.float32))
    ei = np.asarray(edge_index)
    row, col = ei[0].astype(np.int64), ei[1].astype(np.int64)
    deg = (np.bincount(col, minlength=N) + 1).astype(np.float32)
    BIGDEG = np.float32(1e30)   # unused half: 1/sqrt -> ~1e-15 ~= 0

    order = np.argsort(col, kind="stable")
    row_s = row[order]
    starts = np.searchsorted(col[order], np.arange(N + 1))

    # per (core, third): raw jagged lists -> pair-matched slot lists
    thirds = [[None] * N_THIRDS for _ in range(N_CORES)]
    for c in range(N_CORES):
        for t3 in range(N_THIRDS):
            base = _third_base(c, t3)
            sz = THIRD_SIZES[t3]
            tgts = np.arange(base, base + sz)
            cnt = (starts[base + 1:base + sz + 1] - starts[base:base + sz])
            L = cnt + 1                      # + self loop
            Lmax = int(L.max())
            LL = np.full((sz, Lmax), -1, np.int64)
            mask = np.arange(Lmax)[None, :] < cnt[:, None]
            LL[mask] = row_s[starts[base]:starts[base + sz]]
            LL[np.arange(sz), cnt] = tgts    # self loop last

            # --- greedy source-pair matching on the co-occurrence graph ---
            iu, ju = np.triu_indices(Lmax, 1)
            A, B = LL[:, iu], LL[:, ju]
            vm = (A >= 0) & (B >= 0) & (A != B)
            pa = np.minimum(A[vm], B[vm])
            pb = np.maximum(A[vm], B[vm])
            uk, cnts = np.unique(pa * (1 << 20) + pb, return_counts=True)
            uk = uk[np.argsort(-cnts, kind="stable")]
            ua = (uk >> 20).tolist()
            ub = (uk & ((1 << 20) - 1)).tolist()
            partner = {}
            for aa, bb in zip(ua, ub):
                if aa not in partner and bb not in partner:
                    partner[aa] = bb
                    partner[bb] = aa

            # --- per-target slot lists: (rowa, rowb, degA, degB) ---
            slot_ra, slot_rb, slot_da, slot_db = [], [], [], []
            for t in range(sz):
                S = LL[t, :L[t]].tolist()
                used = [False] * len(S)
                pos_of = {}
                for i, si in enumerate(S):
                    pos_of.setdefault(si, []).append(i)
                ra, rb, da, db = [], [], [], []
                for i, si in enumerate(S):
                    if used[i]:
                        continue
                    p = partner.get(si)
                    done = False
                    if p is not None and p in pos_of:
                        for jpos in pos_of[p]:
                            if not used[jpos] and jpos != i:
                                used[i] = used[jpos] = True
                                a, b = (si, p) if si < p else (p, si)
                                ra.append(a)
                                rb.append(b)
                                da.append(deg[a])
                                db.append(deg[b])
                                done = True
                                break
                    if done:
                        continue
                    used[i] = True
                    p = partner.get(si)
                    if p is None:
                        ra.append(si)
                        rb.append(-1)
                        da.append(deg[si])
                        db.append(BIGDEG)
                    else:
                        a, b = (si, p) if si < p else (p, si)
                        ra.append(a)
                        rb.append(b)
                        if si == a:
                            da.append(deg[a])
                            db.append(BIGDEG)
                        else:
                            da.append(BIGDEG)
                            db.append(deg[b])
                slot_ra.append(ra)
                slot_rb.append(rb)
                slot_da.append(da)
                slot_db.append(db)
            Ln = np.array([len(r) for r in slot_ra], np.int64)
            Lnmax = int(Ln.max())
            SRA = np.full((sz, Lnmax), -1, np.int64)
            SRB = np.full((sz, Lnmax), -1, np.int64)
            SDA = np.full((sz, Lnmax), 1.0, np.float32)
            SDB = np.full((sz, Lnmax), 1.0, np.float32)
            for t in range(sz):
                k = Ln[t]
                SRA[t, :k] = slot_ra[t]
                SRB[t, :k] = slot_rb[t]
                SDA[t, :k] = slot_da[t]
                SDB[t, :k] = slot_db[t]
            key = Ln.copy()
            if c == 0 and t3 == 0:
                key[0] = 1 << 40             # node 0 -> rank 0
            rank_order = np.argsort(-key, kind="stable")
            thirds[c][t3] = dict(tgts=tgts, Ln=Ln, SRA=SRA, SRB=SRB,
                                 SDA=SDA, SDB=SDB, rank_order=rank_order)

    # uniform pass widths across cores (per third)
    ws_all, segs_all, C3 = [], [], []
    for t3 in range(N_THIRDS):
        Lmax_g = 0
        for c in range(N_CORES):
            th = thirds[c][t3]
            Lmax_g = max(Lmax_g, int(th["Ln"].max()))
        ws = []
        for d in range(Lmax_g):
            m = 0
            for c in range(N_CORES):
                th = thirds[c][t3]
                Le = th["Ln"][th["rank_order"]].astype(np.int64).copy()
                if c == 0 and t3 == 0:
                    Le[0] = 1 << 40          # node0 occupies every pass
                m = max(m, int((Le > d).sum()))
            ws.append((m + P - 1) // P)
        C = sum(ws)
        Cpad = ((C + CHUNK - 1) // CHUNK) * CHUNK
        ws_all.append(ws)
        C3.append(Cpad)
        cbs = np.cumsum([0] + ws)
        segs = []
        for c0 in range(0, Cpad, CHUNK):
            sl = []
            for d, w in enumerate(ws):
                lo = max(cbs[d], c0)
                hi = min(cbs[d] + w, c0 + CHUNK)
                if lo < hi:
                    sl.append((lo - c0, hi - c0, lo - cbs[d]))
            segs.append(sl)
        segs_all.append(segs)
    CT = sum(C3)

    # per-core slot grids + compact pair tables
    n_tab = 0
    grids = {}
    for c in range(N_CORES):
        for t3 in range(N_THIRDS):
            th = thirds[c][t3]
            ws = ws_all[t3]
            sz = len(th["tgts"])
            ro = th["rank_order"]
            SRAr, SRBr = th["SRA"][ro], th["SRB"][ro]
            SDAr, SDBr = th["SDA"][ro], th["SDB"][ro]
            Lr = th["Ln"][ro].astype(np.int64)
            gkey = np.full((P, C3[t3]), -1, np.int64)
            gda = np.ones((P, C3[t3]), np.float32)
            gdb = np.ones((P, C3[t3]), np.float32)
            cb = 0
            for d, w in enumerate(ws):
                if d < SRAr.shape[1]:
                    rr = np.arange(w * P)
                    pp, cc = rr % P, rr // P
                    valid = (rr < sz) & (d < Lr[np.minimum(rr, sz - 1)])
                    rv = rr[valid]
                    if rv.size:
                        gkey[pp[valid], cb + cc[valid]] = (
                            SRAr[rv, d] * (1 << 20) + (SRBr[rv, d] + 1))
                        gda[pp[valid], cb + cc[valid]] = SDAr[rv, d]
                        gdb[pp[valid], cb + cc[valid]] = SDBr[rv, d]
                cb += w
            uniq = np.unique(gkey[gkey >= 0])
            assert len(uniq) + 1 <= 32767, f"table overflow {len(uniq)}"
            n_tab = max(n_tab, len(uniq) + 1)
            grids[(c, t3)] = (gkey, gda, gdb, uniq)

    in_maps = []
    meta = dict(ws=ws_all, segs=segs_all, C3=C3, CT=CT, n_tab=n_tab,
                node_of_rank=[], deg=deg)
    for c in range(N_CORES):
        m = {}
        idx_cols, dega_cols, degb_cols = [], [], []
        for t3 in range(N_THIRDS):
            gkey, gda, gdb, uniq = grids[(c, t3)]
            idx = np.searchsorted(uniq, np.maximum(gkey, 0))
            idx = np.where(gkey >= 0, idx, n_tab - 1).astype(np.int16)
            ra = (uniq >> 20).astype(np.int64)
            rb = (uniq & ((1 << 20) - 1)).astype(np.int64) - 1
            tab = np.zeros((n_tab, EW), np.float32)
            tab[: len(uniq), :F] = x[ra]
            has_b = rb >= 0
            tab[np.nonzero(has_b)[0], F + 12:2 * F + 12] = x[rb[has_b]]
            m[f"tab{t3}"] = tab
            idx_cols.append(idx)
            dega_cols.append(gda)
            degb_cols.append(gdb)
        idx_full = np.concatenate(idx_cols, axis=1)     # [128, CT]
        blocks = []
        for b in range(CT // CHUNK):
            flat = idx_full[:, b * CHUNK:(b + 1) * CHUNK]
            fj = flat.T.reshape(-1)
            wrapped = fj.reshape(-1, 16).T
            blocks.append(np.tile(wrapped, (8, 1)))
        m["idx"] = np.concatenate(blocks, axis=1).astype(np.int16)
        m["degs"] = np.concatenate(
            [np.concatenate(dega_cols, axis=1),
             np.concatenate(degb_cols, axis=1)], axis=1)  # [128, 2*CT]
        dn = np.ones((P, N_THIRDS * ACOLS), np.float32)
        nor = []
        for t3 in range(N_THIRDS):
            th = thirds[c][t3]
            sz = len(th["tgts"])
            r = np.arange(sz)
            dn[r % P, t3 * ACOLS + r // P] = deg[th["tgts"][th["rank_order"]]]
            nor.append(th["rank_order"])
        m["degn"] = dn
        meta["node_of_rank"].append(nor)
        in_maps.append(m)
    return in_maps, meta


def _build(meta, weights):
    ws_all, segs_all, C3, CT, n_tab = (
        meta["ws"], meta["segs"], meta["C3"], meta["CT"], meta["n_tab"])
    IDXW = CT * 8
    nc = bacc.Bacc("TRN2", target_bir_lowering=False, debug=False,
                   num_devices=N_CORES, num_swdge_queues=NQ)
    tabs = [nc.dram_tensor(f"tab{t}", [n_tab, EW], F32, kind="ExternalInput")
            for t in range(N_THIRDS)]
    idx_in = nc.dram_tensor("idx", [P, IDXW], I16, kind="ExternalInput")
    degs_in = nc.dram_tensor("degs", [P, 2 * CT], F32, kind="ExternalInput")
    degn_in = nc.dram_tensor("degn", [P, N_THIRDS * ACOLS], F32,
                             kind="ExternalInput")
    gw_in = nc.dram_tensor("gw", [F, HID], F32, kind="ExternalInput")
    gamma_in = nc.dram_tensor("gamma", [1, HID], F32, kind="ExternalInput")
    beta_in = nc.dram_tensor("beta", [1, HID], F32, kind="ExternalInput")
    lw_in = nc.dram_tensor("lw", [125, 8], F32, kind="ExternalInput")
    lb_in = nc.dram_tensor("lb", [1, OUT], F32, kind="ExternalInput")
    probs_out = nc.dram_tensor("probs", [P, N_THIRDS * ACOLS * OUT], F32,
                               kind="ExternalOutput")
    rsu_out = nc.dram_tensor("rsu", [1, HID], F32, kind="ExternalOutput")

    with tile.TileContext(nc) as tc:
        with (
            tc.tile_pool(name="const", bufs=1) as const,
            tc.tile_pool(name="gath", bufs=16) as gath,
            tc.tile_pool(name="work", bufs=4) as work,
            tc.tile_pool(name="accp", bufs=1) as accp,
            tc.tile_pool(name="ps", bufs=1, space="PSUM") as ps,
            tc.tile_pool(name="ps2", bufs=2, space="PSUM") as ps2,
            tc.tile_pool(name="dram", bufs=1, space="DRAM") as dram,
        ):
            # warmup: tiny gather to pull the SWDGE gather ucode into IRAM
            wu_idx = const.tile([P, 8], I16)
            nc.vector.memset(wu_idx[:], 0)
            wu_out = const.tile([P, EW], F32)
            nc.gpsimd.dma_gather(
                out_ap=wu_out[:].rearrange("p (c e) -> p c e", c=1),
                in_ap=tabs[0][:], idxs_ap=wu_idx[:], num_idxs=P,
                num_idxs_reg=P, elem_size=EW, single_packet=False,
                queue_num=0)
            # idx tile loaded per third so the first gathers start early
            idx_t = const.tile([P, IDXW], I16)
            degs_t = const.tile([P, 2 * CT], F32)
            ib = 0
            for t3 in range(N_THIRDS):
                w8 = C3[t3] * 8
                nc.sync.dma_start(idx_t[:, ib * 8:ib * 8 + w8],
                                  idx_in[:, ib * 8:ib * 8 + w8])
                nc.sync.dma_start(degs_t[:, ib:ib + C3[t3]],
                                  degs_in[:, ib:ib + C3[t3]])
                nc.sync.dma_start(degs_t[:, CT + ib:CT + ib + C3[t3]],
                                  degs_in[:, CT + ib:CT + ib + C3[t3]])
                ib += C3[t3]
            degn_t = const.tile([P, N_THIRDS * ACOLS], F32)
            nc.sync.dma_start(degn_t[:], degn_in[:])

            # dinv of slot sources (both halves) and of target nodes
            dinv_s = const.tile([P, 2 * CT], F32)
            nc.scalar.activation(dinv_s[:], degs_t[:], AF.Sqrt)
            nc.vector.reciprocal(dinv_s[:], dinv_s[:])
            dinv_n = const.tile([P, N_THIRDS * ACOLS], F32)
            nc.scalar.activation(dinv_n[:], degn_t[:], AF.Sqrt)
            nc.vector.reciprocal(dinv_n[:], dinv_n[:])

            acc3 = []
            for t3 in range(N_THIRDS):
                a = accp.tile([P, ACOLS * (F + 1)], F32, tag=f"acc{t3}")
                nc.vector.memset(a[:], 0.0)
                av = a[:].rearrange("p (c k) -> p c k", k=F + 1)
                nc.vector.memset(av[:, :, F:F + 1], 1.0)
                acc3.append(a)
            ident = const.tile([P, P], F32)
            make_identity(nc, ident[:])

            # ---- gather + scale + JDS accumulate; per-third stats on PE ----
            aT = []            # transposed acc columns for the logits matmuls
            cs3 = []
            qrr = 0
            gcol = 0
            for t3 in range(N_THIRDS):
                av = acc3[t3][:].rearrange("p (c k) -> p c k", k=F + 1)
                for ch in range(C3[t3] // CHUNK):
                    g = gath.tile([P, CHUNK * EW], F32, tag="g")
                    gv = g[:].rearrange("p (c e) -> p c e", c=CHUNK)
                    blk = (gcol // CHUNK) + ch
                    nc.gpsimd.dma_gather(
                        out_ap=gv,
                        in_ap=tabs[t3][:],
                        idxs_ap=idx_t[:, blk * 128:(blk + 1) * 128],
                        num_idxs=CHUNK * P,
                        num_idxs_reg=CHUNK * P,
                        elem_size=EW,
                        single_packet=False,
                        queue_num=qrr % NQ,
                    )
                    qrr += 1
                    sc = work.tile([P, CHUNK * F], F32, tag="sc")
                    scv = sc[:].rearrange("p (c f) -> p c f", c=CHUNK)
                    s2 = work.tile([P, CHUNK * F], F32, tag="s2")
                    s2v = s2[:].rearrange("p (c f) -> p c f", c=CHUNK)
                    c0 = ch * CHUNK
                    dva = dinv_s[:, gcol + c0:gcol + c0 + CHUNK].to_broadcast(
                        [P, CHUNK, F])
                    dvb = dinv_s[:, CT + gcol + c0:CT + gcol + c0
                                 + CHUNK].to_broadcast([P, CHUNK, F])
                    nc.vector.tensor_tensor(
                        out=scv, in0=gv[:, :, 0:F], in1=dva, op=OP.mult)
                    nc.vector.tensor_tensor(
                        out=s2v, in0=gv[:, :, F + 12:2 * F + 12], in1=dvb,
                        op=OP.mult)
                    nc.vector.tensor_tensor(
                        out=scv, in0=scv, in1=s2v, op=OP.add)
                    for (lo, hi, alo) in segs_all[t3][ch]:
                        w = hi - lo
                        nc.vector.tensor_tensor(
                            out=av[:, alo:alo + w, 0:F],
                            in0=av[:, alo:alo + w, 0:F],
                            in1=scv[:, lo:hi, :],
                            op=OP.add)
                gcol += C3[t3]
                # this third's acc is final: scale by target dinv, then
                # stats matmuls + per-column transposes overlap later gathers
                dn = dinv_n[:, t3 * ACOLS:(t3 + 1) * ACOLS].to_broadcast(
                    [P, ACOLS, F])
                nc.vector.tensor_tensor(out=av[:, :, 0:F], in0=av[:, :, 0:F],
                                        in1=dn, op=OP.mult)
                cp = ps.tile([F, F + 1], F32, tag="psC")
                for j in range(ACOLS):
                    nc.tensor.matmul(out=cp[:], lhsT=av[:, j, 0:F],
                                     rhs=av[:, j, 0:F + 1],
                                     start=(j == 0), stop=(j == ACOLS - 1))
                    tps = ps2.tile([F + 1, P], F32, tag="psT")
                    nc.tensor.transpose(out=tps[:], in_=av[:, j, 0:F + 1],
                                        identity=ident[:])
                    ts = const.tile([F + 1, P], F32, tag=f"aT{t3}_{j}")
                    nc.vector.tensor_copy(out=ts[:], in_=tps[:])
                    aT.append(ts)
                cst = const.tile([F, F + 1], F32, tag=f"cs{t3}")
                nc.vector.tensor_copy(out=cst[:], in_=cp[:])
                cs3.append(cst)
            cs = const.tile([F, F + 1], F32)
            nc.vector.tensor_tensor(out=cs[:], in0=cs3[0][:], in1=cs3[1][:],
                                    op=OP.add)
            nc.vector.tensor_tensor(out=cs[:], in0=cs[:], in1=cs3[2][:],
                                    op=OP.add)

            # weights needed only for the epilogue
            gw_t = const.tile([F, HID], F32)
            nc.sync.dma_start(gw_t[:], gw_in[:])
            gamma_t = const.tile([1, HID], F32)
            nc.sync.dma_start(gamma_t[:], gamma_in[:])
            beta_t = const.tile([1, HID], F32)
            nc.sync.dma_start(beta_t[:], beta_in[:])
            lw_t = const.tile([125, 8], F32)
            nc.sync.dma_start(lw_t[:], lw_in[:])
            lb_t = const.tile([1, OUT], F32)
            nc.sync.dma_start(lb_t[:], lb_in[:])

            # ---- AllReduce of C' ----
            bin_ = dram.tile([F, F + 1], F32)
            bout = dram.tile([F, F + 1], F32, addr_space="Shared")
            nc.gpsimd.dma_start(bin_[:], cs[:])
            nc.gpsimd.collective_compute(
                "AllReduce", OP.add,
                replica_groups=[list(range(N_CORES))],
                ins=[bin_[:].opt()], outs=[bout[:].opt()])
            cg = const.tile([F, F + 1], F32)
            nc.sync.dma_start(cg[:], bout[:])

            # ---- fold BN into per-node [21, 2] matrix ----
            cn = const.tile([F, F + 1], F32)
            nc.vector.tensor_scalar_mul(cn[:], cg[:], 1.0 / N)
            gps = ps.tile([F, HID], F32, tag="psA")
            nc.tensor.matmul(out=gps[:], lhsT=cn[:, 0:F], rhs=gw_t[:],
                             start=True, stop=True)
            wg = const.tile([F, HID], F32)
            nc.vector.tensor_tensor(out=wg[:], in0=gw_t[:], in1=gps[:],
                                    op=OP.mult)
            m0ps = ps.tile([1, HID], F32, tag="psB")
            nc.tensor.matmul(out=m0ps[:], lhsT=cn[:, F:F + 1], rhs=gw_t[:],
                             start=True, stop=True)
            m0 = const.tile([1, HID], F32)
            nc.vector.tensor_copy(out=m0[:], in_=m0ps[:])
            ones20 = const.tile([F, 1], F32)
            nc.vector.memset(ones20[:], 1.0)
            e2ps = ps.tile([1, HID], F32, tag="psB")
            nc.tensor.matmul(out=e2ps[:], lhsT=ones20[:], rhs=wg[:],
                             start=True, stop=True)
            var = const.tile([1, HID], F32)
            nc.vector.tensor_tensor(out=var[:], in0=m0[:], in1=m0[:],
                                    op=OP.mult)
            nc.vector.tensor_tensor(out=var[:], in0=e2ps[:], in1=var[:],
                                    op=OP.subtract)
            nc.vector.tensor_scalar_add(var[:], var[:], BN_EPS)
            sd = const.tile([1, HID], F32)
            nc.scalar.activation(sd[:], var[:], AF.Sqrt)
            nc.vector.reciprocal(sd[:], sd[:])
            s_t = const.tile([1, HID], F32)
            nc.vector.tensor_tensor(out=s_t[:], in0=sd[:], in1=gamma_t[:],
                                    op=OP.mult)
            tvec = const.tile([1, HID], F32)
            nc.vector.tensor_tensor(out=tvec[:], in0=m0[:], in1=s_t[:],
                                    op=OP.mult)
            nc.vector.tensor_tensor(out=tvec[:], in0=beta_t[:], in1=tvec[:],
                                    op=OP.subtract)
            ones1x20 = const.tile([1, F], F32)
            nc.vector.memset(ones1x20[:], 1.0)
            sbps = ps.tile([F, HID], F32, tag="psA")
            nc.tensor.matmul(out=sbps[:], lhsT=ones1x20[:], rhs=s_t[:],
                             start=True, stop=True)
            ws_t = const.tile([F, HID], F32)
            nc.vector.tensor_tensor(out=ws_t[:], in0=gw_t[:], in1=sbps[:],
                                    op=OP.mult)
            ident = const.tile([P, P], F32)
            make_identity(nc, ident[:])
            wcbps = ps.tile([F + 1, OUT], F32, tag="psB3")
            for c4 in range(4):
                trp = ps2.tile([125, F], F32, tag="psT")
                nc.tensor.transpose(out=trp[:],
                                    in_=ws_t[:, c4 * 125:(c4 + 1) * 125],
                                    identity=ident[0:F, 0:F])
                trpt = ps2.tile([125, 1], F32, tag="psL")
                nc.tensor.transpose(out=trpt[:],
                                    in_=tvec[:, c4 * 125:(c4 + 1) * 125],
                                    identity=ident[0:1, 0:1])
                trs = work.tile([125, F + 1], F32, tag="trs")
                nc.vector.tensor_copy(out=trs[:, 0:F], in_=trp[:])
                nc.vector.tensor_copy(out=trs[:, F:F + 1], in_=trpt[:])
                nc.tensor.matmul(out=wcbps[:], lhsT=trs[:],
                                 rhs=lw_t[:, c4 * OUT:(c4 + 1) * OUT],
                                 start=(c4 == 0), stop=False)
            e20 = const.tile([1, F + 1], F32)
            nc.vector.memset(e20[:], 0.0)
            nc.vector.memset(e20[:, F:F + 1], 1.0)
            nc.tensor.matmul(out=wcbps[:], lhsT=e20[:], rhs=lb_t[:],
                             start=False, stop=True)
            wcb = const.tile([F + 1, OUT], F32)
            nc.vector.tensor_copy(out=wcb[:], in_=wcbps[:])

            # ---- logits on PE, relu, 2-class softmax ----
            probs_t = const.tile([P, N_THIRDS * ACOLS * OUT], F32)
            pv = probs_t[:].rearrange("p (t c o) -> p t c o", t=N_THIRDS,
                                      o=OUT)
            for t3 in range(N_THIRDS):
                lps = ps2.tile([P, ACOLS * OUT], F32, tag="psL")
                for j in range(ACOLS):
                    nc.tensor.matmul(out=lps[:, j * OUT:(j + 1) * OUT],
                                     lhsT=aT[t3 * ACOLS + j][:],
                                     rhs=wcb[:], start=True, stop=True)
                nc.scalar.activation(pv[:, t3, :, :].rearrange(
                    "p c o -> p (c o)"), lps[:], AF.Relu)
            for t3 in range(N_THIRDS):
                d = work.tile([P, ACOLS], F32, tag="d")
                nc.vector.tensor_tensor(out=d[:], in0=pv[:, t3, :, 0],
                                        in1=pv[:, t3, :, 1], op=OP.subtract)
                nc.scalar.activation(pv[:, t3, :, 0], d[:], AF.Sigmoid)
                nc.scalar.activation(pv[:, t3, :, 1], d[:], AF.Sigmoid,
                                     scale=-1.0)
            nc.sync.dma_start(probs_out[:], probs_t[:])

            # ---- rsu embedding (meaningful on core 0 only) ----
            a0 = acc3[0][0:1, 0:F]                       # node0 at rank 0
            a0ps = ps.tile([F, 1], F32, tag="psB")
            nc.tensor.transpose(out=a0ps[:], in_=a0, identity=ident[0:1, 0:1])
            a0T = work.tile([F, 1], F32, tag="a0T")
            nc.vector.tensor_copy(out=a0T[:], in_=a0ps[:])
            rsups = ps.tile([1, HID], F32, tag="psA")
            nc.tensor.matmul(out=rsups[:], lhsT=a0T[:], rhs=ws_t[:],
                             start=True, stop=True)
            rsu_t = const.tile([1, HID], F32)
            nc.vector.tensor_tensor(out=rsu_t[:], in0=rsups[:], in1=tvec[:],
                                    op=OP.add)
            nc.sync.dma_start(rsu_out[:], rsu_t[:])
    nc.finalize()
    return nc


def _run(inputs, trace=False):
    node_feature = np.asarray(inputs["node_feature"], np.float32)
    edge_index = np.asarray(inputs["edge_index"])
    gcn_W = np.asarray(inputs["gcn_W"], np.float32)
    bn_gamma = np.asarray(inputs["bn_gamma"], np.float32)
    bn_beta = np.asarray(inputs["bn_beta"], np.float32)
    lin_W = np.asarray(inputs["lin_W"], np.float32)
    lin_b = np.asarray(inputs["lin_b"], np.float32)

    in_maps, meta = _preprocess(node_feature, edge_index)
    lw4 = np.ascontiguousarray(
        lin_W.reshape(4, 125, OUT).transpose(1, 0, 2).reshape(125, 4 * OUT))
    for m in in_maps:
        m["gw"] = np.ascontiguousarray(gcn_W)
        m["gamma"] = np.ascontiguousarray(bn_gamma[None, :])
        m["beta"] = np.ascontiguousarray(bn_beta[None, :])
        m["lw"] = lw4
        m["lb"] = np.ascontiguousarray(lin_b[None, :])

    nc = _build(meta, None)
    res = run_bass_kernel_spmd(nc, in_maps, core_ids=list(range(N_CORES)),
                               trace=trace)

    prob_full = np.zeros((N, OUT), np.float32)
    for c in range(N_CORES):
        probs = res.results[c]["probs"].reshape(P, N_THIRDS, ACOLS, OUT)
        for t3 in range(N_THIRDS):
            base = _third_base(c, t3)
            sz = THIRD_SIZES[t3]
            nor = meta["node_of_rank"][c][t3]
            r = np.arange(sz)
            prob_full[base + nor] = probs[r % P, t3, r // P]
    rsu = res.results[0]["rsu"].reshape(1, HID)
    return (prob_full, rsu), res


def kernel(**inputs):
    (prob, rsu), _ = _run(inputs, trace=False)
    return prob, rsu
